# revision 1
# baseline (speedup 1.0000x reference)
"""GCN decoder kernel for Trainium2, 8-core data-parallel over batch.

Per core (one batch sample b):
  Xn = X / max(||X||, 1e-12)                       row-normalize
  S  = Xn @ Xn^T; sig = sigmoid(S - C(1-m_j))      exact-0 masked rows (ACT bias)
  deg = rowsum(sig * m_i) + m;  d = max(deg, 1e-6)^-1/2
  aggT = (m*d)_i * (Y^T @ (sig + diag(m)))  with Y = m*d*X   == (A_norm @ X)^T
         (column mask folded into the (m*d)_i scale -> masked cols exactly 0)
  HfT = relu(W1^T aggT + b1);  PT = W2^T HfT + b2
  out = sigmoid(PT^T PT - C(1-m_j)) * m_i          pair-masked output

Matmul dtypes: S and PPT run in fp16 (full PE rate); W1/W2 chains in f32r.
sig is cached in SBUF as fp16 (16 MiB fp32 would not fit).
"""

from contextlib import ExitStack

import numpy as np

import bass_rust as _bass_rust
import concourse.bass as bass
import concourse.mybir as mybir
import concourse.tile as tile
from concourse.bass_utils import run_bass_kernel_spmd
from concourse.masks import make_identity

F32 = mybir.dt.float32
F16 = mybir.dt.float16
F32R = mybir.dt.float32r
AF = mybir.ActivationFunctionType
OP = mybir.AluOpType

B = 8
N = 2048
D = 256
H = 256
P = 128
NB = N // P  # 16 row blocks
NCH = N // 512  # 4 column chunks of 512
MASK_C = 30000.0


def _install_drain_split(max_waits: int = 1):
    """This walrus build accepts at most ONE sync-wait per instruction.
    (a) split the Tile kernel-tail drain into single-wait drains;
    (b) hoist extra waits from any lowered instruction onto standalone
        EventSemaphore instructions on the same engine."""
    from concourse.vector_clock import ScopedClock

    if getattr(tile.TileContext, "_drain_split_installed", False):
        return

    def _drain_and_barrier(self, tick_clock, wait_clock):
        drain_inst = self.nc.sync.drain()
        wait_clock.add_sem_waits(
            drain_inst.ins, ScopedClock({None: tick_clock.global_clock})
        )
        si = drain_inst.ins.sync_info
        waits = list(si.on_wait) if si is not None and si.on_wait else []
        if len(waits) > max_waits:
            drain_inst.ins.sync_info = _bass_rust.SyncInfo(
                on_wait=waits[:max_waits],
                on_update=list(si.on_update) if si.on_update else [],
            )
            rest = waits[max_waits:]
            for i in range(0, len(rest), max_waits):
                extra = self.nc.sync.drain()
                extra.ins.sync_info = _bass_rust.SyncInfo(
                    on_wait=rest[i : i + max_waits], on_update=[]
                )
        self.nc.all_engine_barrier()
        assert self.sems is not None
        popped = self.nc._tile_sem_poison_stack.pop()
        assert popped is self._sem_poison
        self.nc.clear_and_free_semaphores(list(self.sems.allocated().values()))
        self.nc.all_engine_barrier()

    tile.TileContext._drain_and_barrier = _drain_and_barrier

    orig_add = tile.TileContext._add_instruction
    counter = [0]

    def _add_instruction(self, inst):
        si = inst.sync_info
        if si is not None and si.on_wait and len(si.on_wait) > max_waits:
            waits = list(si.on_wait)
            keep = waits[-max_waits:]
            for w in waits[: -max_waits]:
                counter[0] += 1
                ev = mybir.InstEventSemaphore(
                    name=f"{inst.name}-xw{counter[0]}", ins=[], outs=[]
                )
                ev.engine = inst.engine
                ev.sync_info = _bass_rust.SyncInfo(on_wait=[w], on_update=[])
                orig_add(self, ev)
            inst.sync_info = _bass_rust.SyncInfo(
                on_wait=keep, on_update=list(si.on_update) if si.on_update else []
            )
        orig_add(self, inst)

    tile.TileContext._add_instruction = _add_instruction
    tile.TileContext._drain_split_installed = True


def build_nc(reps=1):
    _install_drain_split()
    nc = bass.Bass("TRN2", target_bir_lowering=False, debug=False, num_devices=B)

    x_d = nc.dram_tensor("x", [N, D], F32, kind="ExternalInput").ap()
    w1_d = nc.dram_tensor("w1", [D, H], F16, kind="ExternalInput").ap()
    w2_d = nc.dram_tensor("w2", [H, H], F16, kind="ExternalInput").ap()
    b1_d = nc.dram_tensor("b1t", [P, H // P], F32, kind="ExternalInput").ap()
    b2_d = nc.dram_tensor("b2t", [P, H // P], F32, kind="ExternalInput").ap()
    mf_d = nc.dram_tensor("mf", [P, NB], F32, kind="ExternalInput").ap()
    rb_d = nc.dram_tensor("rowbias", [P, NB], F32, kind="ExternalInput").ap()
    mr_d = nc.dram_tensor("mrow", [1, N], F16, kind="ExternalInput").ap()
    on_d = nc.dram_tensor("ones16", [1, P], F16, kind="ExternalInput").ap()
    cv_d = nc.dram_tensor("cvec", [P, 1], F32, kind="ExternalInput").ap()
    out_d = nc.dram_tensor("out", [N, N], F32, kind="ExternalOutput").ap()

    with tile.TileContext(nc) as tc:
      for rep in range(reps):
        with ExitStack() as top:
            const = top.enter_context(tc.tile_pool(name=f"const{rep}", bufs=1))
            psum = top.enter_context(
                tc.tile_pool(name=f"psum{rep}", bufs=2, space="PSUM")
            )

            # ---- constants ----
            w1 = const.tile([P, 2, H], F16, tag="w1")  # [d_p, d_chunk, h]
            nc.sync.dma_start(w1[:], w1_d.rearrange("(c p) h -> p c h", p=P))
            w2 = const.tile([P, 2, H], F16, tag="w2")
            nc.sync.dma_start(w2[:], w2_d.rearrange("(c p) h -> p c h", p=P))
            b1v = const.tile([P, 2], F32, tag="b1v")
            nc.sync.dma_start(b1v[:], b1_d[:])
            b2v = const.tile([P, 2], F32, tag="b2v")
            nc.sync.dma_start(b2v[:], b2_d[:])
            mf = const.tile([P, NB], F32, tag="mf")
            nc.sync.dma_start(mf[:], mf_d[:])
            rb = const.tile([P, NB], F32, tag="rb")
            nc.sync.dma_start(rb[:], rb_d[:])
            mrow = const.tile([1, N], F16, tag="mrow")
            nc.sync.dma_start(mrow[:], mr_d[:])
            ones1 = const.tile([1, P], F16, tag="ones1")
            nc.sync.dma_start(ones1[:], on_d[:])
            eye = const.tile([P, P], F32, tag="eye")
            make_identity(nc, eye[:])
            eye16 = const.tile([P, P], F16, tag="eye16")
            make_identity(nc, eye16[:])

            # small per-node vectors in [p, block] layout
            nrm = const.tile([P, NB], F32, tag="nrm")
            dgv = const.tile([P, NB], F32, tag="dgv")
            dga = const.tile([P, NB, 2], F32, tag="dga")
            cvec = const.tile([P, 1], F32, tag="cvec")
            nc.sync.dma_start(cvec[:], cv_d[:])
            dpo = const.tile([P, NB], F32, tag="dpo")
            mdv = const.tile([P, NB], F32, tag="mdv")
            sml = const.tile([P, NB], F32, tag="sml")  # scratch for ln
            d16 = const.tile([16, P], F16, tag="d16")
            drow = const.tile([1, N], F16, tag="drow")
            mbc = const.tile([P, N], F16, tag="mbc")  # column mask, bcast
            dbc = const.tile([P, N], F16, tag="dbc")  # (m*d)_i, bcast

            aggt = const.tile([P, 2, N], F16, tag="aggt")

            # mbc = broadcast of mrow across partitions (rank-1 matmuls)
            pbm = psum.tile([P, N], F32, tag="big")
            for c in range(NCH):
                csl = slice(c * 512, (c + 1) * 512)
                nc.tensor.matmul(
                    pbm[:, csl], ones1[:], mrow[:, csl], start=True, stop=True
                )
            nc.vector.tensor_copy(out=mbc[:], in_=pbm[:])

            with ExitStack() as mid:
                xp = mid.enter_context(tc.tile_pool(name=f"xp{rep}", bufs=1))
                xtp = mid.enter_context(tc.tile_pool(name=f"xtp{rep}", bufs=NB))
                x_sb = [
                    xtp.tile([P, D], F32, tag="x", name=f"x_{rep}_{jb}")
                    for jb in range(NB)
                ]
                y16 = xp.tile([P, NB, D], F16, tag="y16")
                xnt = xp.tile([P, 2, N], F16, tag="xnt")
                sig = xp.tile([P, NB, N], F16, tag="sig")
                tmp = mid.enter_context(tc.tile_pool(name=f"tmp{rep}", bufs=2))

                # ---- phase 0/1: load X, row norms, Xn^T (fp16) ----
                dma_engs = [nc.sync, nc.gpsimd, nc.sync, nc.gpsimd]
                for jb in range(NB):
                    dma_engs[jb % 4].dma_start(
                        x_sb[jb][:], x_d[jb * P : (jb + 1) * P, :]
                    )
                for jb in range(NB):
                    sq = tmp.tile([P, D], F32, tag="sq")
                    nc.scalar.activation(
                        sq[:],
                        x_sb[jb][:],
                        AF.Square,
                        accum_out=nrm[:, jb : jb + 1],
                    )
                # fold column mask into Xn: masked nodes' columns become 0 in
                # xnt, so S has exact-0 there and sigmoid gives 0.5 -> fixed
                # up in deg by the host-provided 0.5*n_masked correction.
                for half in range(2):
                    hs = slice(half * 8, (half + 1) * 8)
                    nc.vector.tensor_scalar_max(nrm[:, hs], nrm[:, hs], 1e-24)
                    nc.scalar.activation(sml[:, hs], nrm[:, hs], AF.Ln)
                    nc.scalar.activation(nrm[:, hs], sml[:, hs], AF.Exp, scale=-0.5)
                    nc.vector.tensor_tensor(
                        nrm[:, hs], nrm[:, hs], mf[:, hs], op=OP.mult
                    )
                for jb in range(NB):
                    xn = tmp.tile([P, D], F16, tag="xn")
                    nc.vector.tensor_scalar_mul(
                        xn[:], x_sb[jb][:], nrm[:, jb : jb + 1]
                    )
                    pt = psum.tile([P, N], F32, tag="big")
                    for k in range(2):
                        pt16 = pt[:, k * 512 : k * 512 + 64].bitcast(F16)
                        nc.tensor.transpose(
                            pt16, xn[:, k * P : (k + 1) * P], eye16[:]
                        )
                    ptb = pt[:].rearrange("p (b r) -> p b r", b=NCH)[
                        :, 0:2, 0:64
                    ].bitcast(F16)
                    nc.vector.tensor_copy(
                        out=xnt[:, :, jb * P : (jb + 1) * P], in_=ptb
                    )

                # ---- phase 2: S = Xn Xn^T, sigmoid w/ row mask, deg ----
                for jb in range(NB):
                    jsl = slice(jb * P, (jb + 1) * P)
                    ps = psum.tile([P, N], F32, tag="big")
                    for k in range(2):
                        for c in range(NCH):
                            csl = slice(c * 512, (c + 1) * 512)
                            nc.tensor.matmul(
                                ps[:, csl],
                                xnt[:, k, jsl],
                                xnt[:, k, csl],
                                start=(k == 0),
                                stop=(k == 1),
                            )
                    for hh in range(2):
                        hsl2 = slice(hh * 1024, (hh + 1) * 1024)
                        nc.scalar.activation(
                            sig[:, jb, hsl2],
                            ps[:, hsl2],
                            AF.Sigmoid,
                            bias=rb[:, jb : jb + 1],
                            accum_out=dga[:, jb, hh : hh + 1],
                        )
                    # add diag(m) into the diagonal block (after deg accum)
                    nc.vector.scalar_tensor_tensor(
                        out=sig[:, jb, jsl],
                        in0=eye[:],
                        scalar=mf[:, jb : jb + 1],
                        in1=sig[:, jb, jsl],
                        op0=OP.mult,
                        op1=OP.add,
                    )

                # ---- phase 3: d = max(deg + m, eps)^-1/2, Y = m*d*X, dbc ----
                nc.vector.tensor_reduce(
                    out=dgv[:], in_=dga[:], axis=mybir.AxisListType.X, op=OP.add
                )
                nc.vector.tensor_tensor(dgv[:], dgv[:], mf[:], op=OP.add)
                nc.vector.tensor_scalar_sub(dgv[:], dgv[:], cvec[:, 0:1])
                nc.vector.tensor_scalar_max(dgv[:], dgv[:], 1e-6)
                nc.scalar.activation(sml[:], dgv[:], AF.Ln)
                nc.scalar.activation(dpo[:], sml[:], AF.Exp, scale=-0.5)
                nc.vector.tensor_tensor(mdv[:], mf[:], dpo[:], op=OP.mult)
                # dbc[p, i] = (m*d)_i for every partition p:
                # transpose mdv -> [16, 128] fp16, then 16 rank-1 broadcasts
                ptd = psum.tile([P, N], F32, tag="big")
                nc.tensor.transpose(ptd[0:16, 0:P], mdv[:], eye[:])
                nc.vector.tensor_copy(out=d16[:], in_=ptd[0:16, 0:P])
                nc.sync.dma_start(
                    drow[0:1].rearrange("p (o q) -> p o q", o=16), d16[:]
                )
                pbd = psum.tile([P, N], F32, tag="big")
                for c in range(NCH):
                    csl = slice(c * 512, (c + 1) * 512)
                    nc.tensor.matmul(
                        pbd[:, csl], ones1[:], drow[0:1, csl], start=True, stop=True
                    )
                nc.vector.tensor_copy(out=dbc[:], in_=pbd[:])
                for jb in range(NB):
                    nc.vector.tensor_scalar_mul(
                        y16[:, jb, :], x_sb[jb][:], mdv[:, jb : jb + 1]
                    )

                # ---- phase 4: aggT = dbc * (Y^T (sig + diag(m))) ----
                for ig in range(NCH):
                    isl = slice(ig * 512, (ig + 1) * 512)
                    ps = psum.tile([P, N], F32, tag="big")
                    for jb in range(NB):
                        nc.tensor.matmul(
                            ps[:, 0:512],
                            y16[:, jb, 0:P],
                            sig[:, jb, isl],
                            start=(jb == 0),
                            stop=(jb == NB - 1),
                        )
                        nc.tensor.matmul(
                            ps[:, 512:1024],
                            y16[:, jb, P : 2 * P],
                            sig[:, jb, isl],
                            start=(jb == 0),
                            stop=(jb == NB - 1),
                        )
                    nc.vector.tensor_tensor(
                        aggt[:, 0, isl], ps[:, 0:512], dbc[:, isl], op=OP.mult
                    )
                    nc.vector.tensor_tensor(
                        aggt[:, 1, isl], ps[:, 512:1024], dbc[:, isl], op=OP.mult
                    )

            # ---- phase 5: HfT = relu(W1^T aggT + b1), PT = W2^T HfT + b2 ----
            with ExitStack() as bot:
                hp = bot.enter_context(tc.tile_pool(name=f"hp{rep}", bufs=1))
                outp = bot.enter_context(tc.tile_pool(name=f"outp{rep}", bufs=6))
                hft = hp.tile([P, 2, N], F16, tag="hft")
                ptt = hp.tile([P, 2, N], F16, tag="ptt")
                for hb in range(2):
                    hsl = slice(hb * P, (hb + 1) * P)
                    ps = psum.tile([P, N], F32, tag="big")
                    for k in range(2):
                        for c in range(NCH):
                            csl = slice(c * 512, (c + 1) * 512)
                            nc.tensor.matmul(
                                ps[:, csl],
                                w1[:, k, hsl],
                                aggt[:, k, csl],
                                start=(k == 0),
                                stop=(k == 1),
                            )
                    for hh in range(2):
                        hsl2 = slice(hh * 1024, (hh + 1) * 1024)
                        nc.scalar.activation(
                            hft[:, hb, hsl2],
                            ps[:, hsl2],
                            AF.Relu,
                            bias=b1v[:, hb : hb + 1],
                        )
                pps = [
                    psum.tile([P, N], F32, tag="big", name=f"pps{rep}_{hb}")
                    for hb in range(2)
                ]
                for k in range(2):
                    for hb in range(2):
                        hsl = slice(hb * P, (hb + 1) * P)
                        for c in range(NCH):
                            csl = slice(c * 512, (c + 1) * 512)
                            nc.tensor.matmul(
                                pps[hb][:, csl],
                                w2[:, k, hsl],
                                hft[:, k, csl],
                                start=(k == 0),
                                stop=(k == 1),
                            )
                for hb in range(2):
                    for hh in range(2):
                        hsl2 = slice(hh * 1024, (hh + 1) * 1024)
                        nc.scalar.activation(
                            ptt[:, hb, hsl2],
                            pps[hb][:, hsl2],
                            AF.Identity,
                            bias=b2v[:, hb : hb + 1],
                        )

                # ---- phase 6: out = sigmoid(PT^T PT + row bias) * m_i ----
                for jb in range(NB):
                    jsl = slice(jb * P, (jb + 1) * P)
                    ps = psum.tile([P, N], F32, tag="big")
                    for k in range(2):
                        for c in range(NCH):
                            csl = slice(c * 512, (c + 1) * 512)
                            nc.tensor.matmul(
                                ps[:, csl],
                                ptt[:, k, jsl],
                                ptt[:, k, csl],
                                start=(k == 0),
                                stop=(k == 1),
                            )
                    osb = outp.tile([P, N], F32, tag="osb")
                    for hh in range(2):
                        hsl2 = slice(hh * 1024, (hh + 1) * 1024)
                        nc.scalar.activation(
                            osb[:, hsl2],
                            ps[:, hsl2],
                            AF.Sigmoid,
                            bias=rb[:, jb : jb + 1],
                        )
                    nc.vector.tensor_tensor(osb[:], osb[:], mbc[:], op=OP.mult)
                    [nc.sync, nc.scalar, nc.gpsimd, nc.sync][jb % 3].dma_start(
                        out_d[jsl, :], osb[:]
                    )

    return nc


def _round_fp22(a):
    """Round fp32 values to FP22 (e8m13) nearest — what the PE uses for f32r."""
    u = np.ascontiguousarray(a, dtype=np.float32).view(np.uint32)
    r = ((u.astype(np.uint64) + 0x200) & 0xFFFFFC00).astype(np.uint32)
    return r.view(np.float32).reshape(a.shape)


_NC_CACHE = None


def _get_nc():
    global _NC_CACHE
    if _NC_CACHE is None:
        _NC_CACHE = build_nc()
    return _NC_CACHE


def make_in_maps(X, mask, W1, b1, W2, b2):
    X = np.asarray(X, dtype=np.float32)
    mask = np.asarray(mask)
    W1 = np.asarray(W1, dtype=np.float32)
    b1 = np.asarray(b1, dtype=np.float32)
    W2 = np.asarray(W2, dtype=np.float32)
    b2 = np.asarray(b2, dtype=np.float32)

    b1t = np.ascontiguousarray(b1.reshape(H // P, P).T)
    b2t = np.ascontiguousarray(b2.reshape(H // P, P).T)
    in_maps = []
    for b in range(B):
        m = mask[b].astype(np.float32)
        bias = -MASK_C * (1.0 - m)
        in_maps.append(
            {
                "x": np.ascontiguousarray(X[b]),
                "w1": W1.astype(np.float16),
                "w2": W2.astype(np.float16),
                "b1t": b1t,
                "b2t": b2t,
                "mf": np.ascontiguousarray(m.reshape(NB, P).T),
                "rowbias": np.ascontiguousarray(bias.reshape(NB, P).T),
                "mrow": m.reshape(1, N).astype(np.float16),
                "ones16": np.ones((1, P), dtype=np.float16),
                "cvec": np.full((P, 1), 0.5 * float(N - m.sum()), dtype=np.float32),
            }
        )
    return in_maps


def kernel(X, mask, W1, b1, W2, b2):
    nc = _get_nc()
    in_maps = make_in_maps(X, mask, W1, b1, W2, b2)
    res = run_bass_kernel_spmd(nc, in_maps, list(range(B)))
    out = np.stack([res.results[b]["out"] for b in range(B)], axis=0)
    return out.astype(np.float32)



# revision 4
# speedup vs baseline: 2.5243x; 2.5243x over previous
"""GCN decoder kernel for Trainium2, 8-core data-parallel over batch.

Key idea: the mask is random 0/1, so only K~=1024 of 2048 nodes are active
per sample, and all masked rows/cols of the output are exactly zero (pair
mask). The computation is permutation-equivariant, so the host packs the
active nodes first (padded to KN = 128*ceil(Kmax/128)) and the device runs
the whole pipeline on [KN, KN] ~= 1/4 of the work. The host scatters the
[K, K] result back into a zero [N, N] matrix.

Host precomputes Xn = X/||X|| (shipped as XnT fp16) so the device does:
  S   = XnT^T XnT;  sig = sigmoid(S) + diag(1)      (f16, ACT)
  deg = rowsum(sig) - 0.5*(KN-K);  d = rsqrt(max(deg, 1e-6))   (DVE,
        rsqrt via bit-hack + 3 Newton steps -- no ACT table switch)
  aggT = d_i * (Y^T @ sig),  Y = d*X (f16)          == (A_norm @ X)^T
  HfT = relu(W1^T aggT + b1);  PT = W2^T HfT + b2   (DVE bias/relu)
  out = sigmoid(PT^T PT)  (f16)  -> host slices [K, K] and scatters.

Padded rows have X=0 -> Xn=0 -> S cols exact 0 -> sigmoid 0.5, corrected in
deg by the host-provided 0.5*(KN-K); padded rows of Y are 0 so they never
contribute; padded output rows/cols are discarded on host.
"""

from contextlib import ExitStack

import numpy as np

import bass_rust as _bass_rust
import concourse.bass as bass
import concourse.mybir as mybir
import concourse.tile as tile
from concourse.bass_utils import run_bass_kernel_spmd
from concourse.masks import make_identity

F32 = mybir.dt.float32
F16 = mybir.dt.float16
I32 = mybir.dt.int32
AF = mybir.ActivationFunctionType
OP = mybir.AluOpType

B = 8
N = 2048
D = 256
H = 256
P = 128
MAGIC1 = 0x5F3759DF + 1  # fp32 rsqrt bit-hack constant (+1 for the xor trick)


def _install_drain_split(max_waits: int = 1):
    """This walrus build accepts at most ONE sync-wait per instruction.
    (a) split the Tile kernel-tail drain into single-wait drains;
    (b) hoist extra waits from any lowered instruction onto standalone
    EventSemaphore instructions on the same engine."""
    from concourse.vector_clock import ScopedClock

    if getattr(tile.TileContext, "_drain_split_installed", False):
        return

    def _drain_and_barrier(self, tick_clock, wait_clock):
        drain_inst = self.nc.sync.drain()
        wait_clock.add_sem_waits(
            drain_inst.ins, ScopedClock({None: tick_clock.global_clock})
        )
        si = drain_inst.ins.sync_info
        waits = list(si.on_wait) if si is not None and si.on_wait else []
        if len(waits) > max_waits:
            drain_inst.ins.sync_info = _bass_rust.SyncInfo(
                on_wait=waits[:max_waits],
                on_update=list(si.on_update) if si.on_update else [],
            )
            rest = waits[max_waits:]
            for i in range(0, len(rest), max_waits):
                extra = self.nc.sync.drain()
                extra.ins.sync_info = _bass_rust.SyncInfo(
                    on_wait=rest[i : i + max_waits], on_update=[]
                )
        self.nc.all_engine_barrier()
        assert self.sems is not None
        popped = self.nc._tile_sem_poison_stack.pop()
        assert popped is self._sem_poison
        self.nc.clear_and_free_semaphores(list(self.sems.allocated().values()))
        self.nc.all_engine_barrier()

    tile.TileContext._drain_and_barrier = _drain_and_barrier

    orig_add = tile.TileContext._add_instruction
    counter = [0]

    def _add_instruction(self, inst):
        si = inst.sync_info
        if si is not None and si.on_wait and len(si.on_wait) > max_waits:
            waits = list(si.on_wait)
            keep = waits[-max_waits:]
            for w in waits[: -max_waits]:
                counter[0] += 1
                ev = mybir.InstEventSemaphore(
                    name=f"{inst.name}-xw{counter[0]}", ins=[], outs=[]
                )
                ev.engine = inst.engine
                ev.sync_info = _bass_rust.SyncInfo(on_wait=[w], on_update=[])
                orig_add(self, ev)
            inst.sync_info = _bass_rust.SyncInfo(
                on_wait=keep, on_update=list(si.on_update) if si.on_update else []
            )
        orig_add(self, inst)

    tile.TileContext._add_instruction = _add_instruction
    tile.TileContext._drain_split_installed = True


def build_nc(kb: int):
    _install_drain_split()
    KN = P * kb
    KNP = max(KN, 1024)  # psum tile width (>= 1024 for the agg 2-half layout)
    chunks = [(c, min(512, KN - c)) for c in range(0, KN, 512)]

    nc = bass.Bass("TRN2", target_bir_lowering=False, debug=False, num_devices=B)

    xnt_d = nc.dram_tensor("xnt", [P, 2, KN], F16, kind="ExternalInput").ap()
    x16_d = nc.dram_tensor("x16", [KN, D], F16, kind="ExternalInput").ap()
    w1_d = nc.dram_tensor("w1", [D, H], F16, kind="ExternalInput").ap()
    w2_d = nc.dram_tensor("w2", [H, H], F16, kind="ExternalInput").ap()
    b1_d = nc.dram_tensor("b1t", [P, H // P], F32, kind="ExternalInput").ap()
    b2_d = nc.dram_tensor("b2t", [P, H // P], F32, kind="ExternalInput").ap()
    cv_d = nc.dram_tensor("cvec", [P, 1], F32, kind="ExternalInput").ap()
    on_d = nc.dram_tensor("ones16", [1, P], F16, kind="ExternalInput").ap()
    out_d = nc.dram_tensor("out", [KN, KN], F16, kind="ExternalOutput").ap()

    with tile.TileContext(nc) as tc:
        with ExitStack() as top:
            const = top.enter_context(tc.tile_pool(name="const", bufs=1))
            psum = top.enter_context(tc.tile_pool(name="psum", bufs=2, space="PSUM"))
            outp = top.enter_context(tc.tile_pool(name="outp", bufs=3))

            # ---- input DMAs first (xnt gates phase 2) ----
            xnt = const.tile([P, 2, KN], F16, tag="xnt")
            nc.sync.dma_start(xnt[:], xnt_d[:])
            x16 = const.tile([P, kb, D], F16, tag="x16")
            nc.gpsimd.dma_start(x16[:], x16_d.rearrange("(b p) d -> p b d", p=P))
            w1 = const.tile([P, 2, H], F16, tag="w1")
            nc.gpsimd.dma_start(w1[:], w1_d.rearrange("(c p) h -> p c h", p=P))
            w2 = const.tile([P, 2, H], F16, tag="w2")
            nc.gpsimd.dma_start(w2[:], w2_d.rearrange("(c p) h -> p c h", p=P))
            b1v = const.tile([P, 2], F32, tag="b1v")
            nc.sync.dma_start(b1v[:], b1_d[:])
            b2v = const.tile([P, 2], F32, tag="b2v")
            nc.sync.dma_start(b2v[:], b2_d[:])
            cvec = const.tile([P, 1], F32, tag="cvec")
            nc.sync.dma_start(cvec[:], cv_d[:])
            ones1 = const.tile([1, P], F16, tag="ones1")
            nc.sync.dma_start(ones1[:], on_d[:])

            # warm up the ACT sigmoid table set right away
            warm = const.tile([1, 2], F32, tag="warm")
            nc.vector.memset(warm[0:1, 0:1], 0.0)
            nc.scalar.activation(warm[0:1, 1:2], warm[0:1, 0:1], AF.Sigmoid)

            eye = const.tile([P, P], F32, tag="eye")
            make_identity(nc, eye[:])

            sig = const.tile([P, kb, KN], F16, tag="sig")
            y16 = const.tile([P, kb, D], F16, tag="y16")
            aggt = const.tile([P, 2, KN], F16, tag="aggt")
            hft = const.tile([P, 2, KN], F16, tag="hft")
            ptt = const.tile([P, 2, KN], F16, tag="ptt")
            dbc = const.tile([P, KN], F16, tag="dbc")
            drow = const.tile([1, KN], F16, tag="drow")
            d16 = const.tile([16, P], F16, tag="d16")
            dgv = const.tile([P, kb], F32, tag="dgv")
            dxh = const.tile([P, kb], F32, tag="dxh")
            dsy = const.tile([P, kb], F32, tag="dsy")
            dnt = const.tile([P, kb], F32, tag="dnt")

            # ---- phase 2: S = Xn Xn^T, sigmoid (f16), diag, rowsum ----
            for jb in range(kb):
                jsl = slice(jb * P, (jb + 1) * P)
                ps = psum.tile([P, KNP], F32, tag="big")
                for k in range(2):
                    for c0, cw in chunks:
                        nc.tensor.matmul(
                            ps[:, c0 : c0 + cw],
                            xnt[:, k, jsl],
                            xnt[:, k, c0 : c0 + cw],
                            start=(k == 0),
                            stop=(k == 1),
                        )
                nc.scalar.activation(sig[:, jb, :], ps[:, 0:KN], AF.Sigmoid)
                # add identity on the diagonal block (self loops, active rows;
                # padded rows' +1 only touches discarded columns)
                nc.vector.scalar_tensor_tensor(
                    out=sig[:, jb, jsl],
                    in0=eye[:],
                    scalar=1.0,
                    in1=sig[:, jb, jsl],
                    op0=OP.mult,
                    op1=OP.add,
                )
                nc.vector.tensor_reduce(
                    out=dgv[:, jb : jb + 1],
                    in_=sig[:, jb, :],
                    axis=mybir.AxisListType.X,
                    op=OP.add,
                )

            # ---- phase 3: d = rsqrt(max(deg - cvec, 1e-6)), Y, dbc ----
            nc.vector.tensor_scalar(
                out=dgv[:],
                in0=dgv[:],
                scalar1=cvec[:, 0:1],
                scalar2=1e-6,
                op0=OP.subtract,
                op1=OP.max,
            )
            nc.vector.tensor_scalar_mul(dxh[:], dgv[:], -0.5)  # -x/2
            # y0 bits = MAGIC - (i >> 1)  ==  ((i >> 1) ^ -1) + (MAGIC + 1)
            nc.vector.tensor_scalar(
                out=dsy[:].bitcast(I32),
                in0=dgv[:].bitcast(I32),
                scalar1=1,
                scalar2=-1,
                op0=OP.logical_shift_right,
                op1=OP.bitwise_xor,
            )
            nc.vector.tensor_scalar_add(
                dsy[:].bitcast(I32), dsy[:].bitcast(I32), MAGIC1
            )
            for _ in range(3):  # Newton: y *= 1.5 - 0.5*x*y^2
                nc.vector.tensor_tensor(dnt[:], dsy[:], dsy[:], op=OP.mult)
                nc.vector.tensor_tensor(dnt[:], dnt[:], dxh[:], op=OP.mult)
                nc.vector.tensor_scalar_add(dnt[:], dnt[:], 1.5)
                nc.vector.tensor_tensor(dsy[:], dsy[:], dnt[:], op=OP.mult)
            for jb in range(kb):
                nc.vector.tensor_scalar_mul(
                    y16[:, jb, :], x16[:, jb, :], dsy[:, jb : jb + 1]
                )
            # dbc[p, i] = d_i for all p: transpose d, then rank-1 broadcasts
            ptd = psum.tile([P, KNP], F32, tag="big")
            nc.tensor.transpose(ptd[0:kb, 0:P], dsy[:], eye[:])
            nc.vector.tensor_copy(out=d16[0:kb, :], in_=ptd[0:kb, 0:P])
            nc.sync.dma_start(
                drow[0:1].rearrange("p (o q) -> p o q", o=kb), d16[0:kb, :]
            )
            pbd = psum.tile([P, KNP], F32, tag="big")
            for c0, cw in chunks:
                nc.tensor.matmul(
                    pbd[:, c0 : c0 + cw],
                    ones1[:],
                    drow[0:1, c0 : c0 + cw],
                    start=True,
                    stop=True,
                )
            nc.vector.tensor_copy(out=dbc[:], in_=pbd[:, 0:KN])

            # ---- phase 4: aggT = dbc * (Y^T @ sig) ----
            for c0, cw in chunks:
                ps = psum.tile([P, KNP], F32, tag="big")
                for jb in range(kb):
                    nc.tensor.matmul(
                        ps[:, 0:cw],
                        y16[:, jb, 0:P],
                        sig[:, jb, c0 : c0 + cw],
                        start=(jb == 0),
                        stop=(jb == kb - 1),
                    )
                    nc.tensor.matmul(
                        ps[:, 512 : 512 + cw],
                        y16[:, jb, P : 2 * P],
                        sig[:, jb, c0 : c0 + cw],
                        start=(jb == 0),
                        stop=(jb == kb - 1),
                    )
                nc.vector.tensor_tensor(
                    aggt[:, 0, c0 : c0 + cw],
                    ps[:, 0:cw],
                    dbc[:, c0 : c0 + cw],
                    op=OP.mult,
                )
                nc.vector.tensor_tensor(
                    aggt[:, 1, c0 : c0 + cw],
                    ps[:, 512 : 512 + cw],
                    dbc[:, c0 : c0 + cw],
                    op=OP.mult,
                )

            # ---- phase 5: HfT = relu(W1^T aggT + b1), PT = W2^T HfT + b2 ----
            for hb in range(2):
                hsl = slice(hb * P, (hb + 1) * P)
                ps = psum.tile([P, KNP], F32, tag="big")
                for k in range(2):
                    for c0, cw in chunks:
                        nc.tensor.matmul(
                            ps[:, c0 : c0 + cw],
                            w1[:, k, hsl],
                            aggt[:, k, c0 : c0 + cw],
                            start=(k == 0),
                            stop=(k == 1),
                        )
                nc.vector.tensor_scalar(
                    out=hft[:, hb, :],
                    in0=ps[:, 0:KN],
                    scalar1=b1v[:, hb : hb + 1],
                    scalar2=0.0,
                    op0=OP.add,
                    op1=OP.max,
                )
            for hb in range(2):
                hsl = slice(hb * P, (hb + 1) * P)
                ps = psum.tile([P, KNP], F32, tag="big")
                for k in range(2):
                    for c0, cw in chunks:
                        nc.tensor.matmul(
                            ps[:, c0 : c0 + cw],
                            w2[:, k, hsl],
                            hft[:, k, c0 : c0 + cw],
                            start=(k == 0),
                            stop=(k == 1),
                        )
                nc.vector.tensor_scalar_add(
                    ptt[:, hb, :], ps[:, 0:KN], b2v[:, hb : hb + 1]
                )

            # ---- phase 6: out = sigmoid(PT^T PT) (f16), DMA out ----
            dma_engs = [nc.sync, nc.gpsimd]
            for jb in range(kb):
                jsl = slice(jb * P, (jb + 1) * P)
                ps = psum.tile([P, KNP], F32, tag="big")
                for k in range(2):
                    for c0, cw in chunks:
                        nc.tensor.matmul(
                            ps[:, c0 : c0 + cw],
                            ptt[:, k, jsl],
                            ptt[:, k, c0 : c0 + cw],
                            start=(k == 0),
                            stop=(k == 1),
                        )
                osb = outp.tile([P, KN], F16, tag="osb")
                nc.scalar.activation(osb[:], ps[:, 0:KN], AF.Sigmoid)
                dma_engs[jb % 2].dma_start(out_d[jsl, :], osb[:])

    return nc


_NC_CACHE: dict[int, object] = {}


def _get_nc(kb: int):
    if kb not in _NC_CACHE:
        _NC_CACHE[kb] = build_nc(kb)
    return _NC_CACHE[kb]


def _plan(mask):
    """Active indices per sample and the shared padded size KN."""
    idxs = [np.nonzero(np.asarray(mask[b]) != 0)[0] for b in range(mask.shape[0])]
    kmax = max((len(i) for i in idxs), default=1)
    kb = max(1, -(-kmax // P))
    return idxs, kb


def make_in_maps(X, mask, W1, b1, W2, b2, idxs, kb):
    KN = P * kb
    X = np.asarray(X, dtype=np.float32)
    W1 = np.asarray(W1, dtype=np.float32)
    b1 = np.asarray(b1, dtype=np.float32)
    W2 = np.asarray(W2, dtype=np.float32)
    b2 = np.asarray(b2, dtype=np.float32)

    b1t = np.ascontiguousarray(b1.reshape(H // P, P).T)
    b2t = np.ascontiguousarray(b2.reshape(H // P, P).T)
    w1h = W1.astype(np.float16)
    w2h = W2.astype(np.float16)
    ones = np.ones((1, P), dtype=np.float16)
    in_maps = []
    for b in range(B):
        idx = idxs[b]
        K = len(idx)
        Xp = np.zeros((KN, D), dtype=np.float32)
        Xp[:K] = X[b][idx]
        nrm = np.maximum(np.linalg.norm(Xp, axis=1, keepdims=True), 1e-12)
        Xn = Xp / nrm
        xnt = np.ascontiguousarray(
            Xn.T.reshape(2, P, KN).transpose(1, 0, 2)
        ).astype(np.float16)
        in_maps.append(
            {
                "xnt": xnt,
                "x16": Xp.astype(np.float16),
                "w1": w1h,
                "w2": w2h,
                "b1t": b1t,
                "b2t": b2t,
                "cvec": np.full((P, 1), 0.5 * float(KN - K), dtype=np.float32),
                "ones16": ones,
            }
        )
    return in_maps


def kernel(X, mask, W1, b1, W2, b2):
    mask = np.asarray(mask)
    idxs, kb = _plan(mask)
    nc = _get_nc(kb)
    in_maps = make_in_maps(X, mask, W1, b1, W2, b2, idxs, kb)
    res = run_bass_kernel_spmd(nc, in_maps, list(range(B)))
    out = np.zeros((B, N, N), dtype=np.float32)
    for b in range(B):
        idx = idxs[b]
        K = len(idx)
        if K:
            o = np.asarray(res.results[b]["out"])[:K, :K].astype(np.float32)
            out[b][np.ix_(idx, idx)] = o
    return out
